# revision 15
# baseline (speedup 1.0000x reference)
"""Trainium2 Bass kernel for nn_NetV2 sparse CNN (submanifold sparse conv net).

Network: scatter 150 active pixels/image to 28x28 grid -> SubMConv3x3(1->32)+BN+ReLU
-> SubMConv3x3(32->64)+BN+ReLU -> SparseConv2x2s2(64->64)+BN+ReLU -> flatten(NCHW)
-> FC(12544->128)+ReLU -> FC(128->10) -> log_softmax.

Design notes:
  * The active-pixel pattern is identical for every image, so each sparse conv
    layer is a fixed gather+matmul structure shared batch-wide.
  * Batch is data-parallel across 8 cores (256 images/core) and lives on the
    matmul free (N) axis; channels/sites live on partitions. BN folds into
    conv weights+bias.
  * H1S blocks hold conv1 outputs as 4 x 32-channel site slots per 128-row
    block, packed with SLOT SHARING: a greedy set-cover places sites so that
    every output site's <=4 active neighbors live in ONE shared block (46
    blocks / 181 slots vs 338 slots for naive per-consumer stacking).  Each
    block is produced by a single windowed conv1 matmul (operator columns
    emit the site's 3x3-neighborhood combination), so conv2 is one K=128
    matmul per site into its PSUM half; n=5 sites accumulate one extra
    matmul against whichever block holds the 5th neighbor.
  * conv3 cells read stacked site pairs from H2S blocks; single-site cells
    use per-(half,offset) weight variants. All matmuls use tile_position row
    base 0, so PSUM banks can be packed freely.
  * PSUM is organized as 3 double-bank supertiles [128,1024]: 4 column
    groups of matmuls per supertile, evacuated by ONE fused relu+bias op
    alternating between the scalar and vector engines (PSUM reads are the
    1 elem/cycle/lane bottleneck; bigger ops amortize ~200ns/op overhead).
  * Input DMA is spread across the sync/scalar/gpsimd descriptor queues so
    xc+t1p land ASAP; small parameters are batched into one transfer.
  * Dummy warm-up matmuls + ACT-table preloads run during the DMA window so
    the PE HAM clock ramps to 2.4 GHz while inputs land.
  * FC1 only needs rows of fc1_w at active output cells (117 of 196).
  * log_softmax skips the max-subtraction: |logits| < 0.5 by construction
    of the fixed input distribution.

All matmul operands are bf16 (fp32 PSUM accumulate); rel err ~2e-4 vs fp32 ref.
"""

import numpy as np
import ml_dtypes

B = 2048
S = 150          # active sites per image
H = W = 28
NCORES = 8
BC = B // NCORES  # batch per core = 256
EPS = 1e-5
BF = ml_dtypes.bfloat16
NWARM = 16       # PE warm-up matmuls


_CACHE = {}


# ---------------------------------------------------------------- metadata ---

def _build_meta(yy, xx):
    """Site graph + placement metadata from the shared active-pixel pattern."""
    order = np.argsort(yy.astype(np.int64) * W + xx)  # row-major spatial sort
    yy_s, xx_s = yy[order], xx[order]
    grid = -np.ones((H, W), np.int64)
    grid[yy_s, xx_s] = np.arange(S)

    # 3x3 pad-1 neighbor lists: per out site i, list of (k, j)
    nbrs = []
    for i in range(S):
        y, x = int(yy_s[i]), int(xx_s[i])
        lst = []
        for ky in range(3):
            for kx in range(3):
                iy, ix = y + ky - 1, x + kx - 1
                if 0 <= iy < H and 0 <= ix < W and grid[iy, ix] >= 0:
                    lst.append((ky * 3 + kx, int(grid[iy, ix])))
        nbrs.append(lst)

    def contributors(j):
        # conv1 column for site j sums over j's own 3x3 active neighbors
        return [j2 for _, j2 in nbrs[j]]

    def window_ok(sites):
        cont = set()
        for j in sites:
            cont.update(contributors(j))
        lo, hi = min(cont), max(cont)
        return (hi // 32 - lo // 32) * 32 + 32 <= 128

    # ---- H1S shared-slot packing -------------------------------------------
    # Bins of <=4 sites; every output site's prim4 neighbor set must fit in
    # one bin whose conv1 contributor-union spans <=128 sorted-site window.
    prim4 = [frozenset(j for _, j in nbrs[i][:min(len(nbrs[i]), 4)])
             for i in range(S)]
    import random as _random
    rnd = _random.Random(0)

    def pack(site_order):
        bins, assign = [], {}
        for i in site_order:
            sset = prim4[i]
            hit = False
            for b, bset in enumerate(bins):
                if sset <= bset:
                    assign[i] = b
                    hit = True
                    break
            if hit:
                continue
            best = None
            for b, bset in enumerate(bins):
                u = bset | sset
                if len(u) <= 4 and window_ok(u):
                    cost = len(u) - len(bset)
                    if best is None or cost < best[0]:
                        best = (cost, b, u)
            if best is not None:
                _, b, u = best
                bins[b] = u
                assign[i] = b
            else:
                bins.append(set(sset))
                assign[i] = len(bins) - 1
        return bins, assign

    best = None
    orders = [sorted(range(S), key=lambda i: (-len(prim4[i]), i))]
    for _ in range(30):
        orders.append(sorted(range(S), key=lambda i: (-len(prim4[i]),
                                                      rnd.random())))
    for o in orders:
        bins, assign = pack(o)
        if best is None or len(bins) < best[0]:
            best = (len(bins), bins, assign)
    nb1, bins, assign = best
    binsites = [sorted(bset) for bset in bins]      # slot s -> site
    binslot = [{j: s for s, j in enumerate(bs)} for bs in binsites]
    site_any = {}                                   # site -> (bin, slot)
    for b, bs in enumerate(binsites):
        for s, j in enumerate(bs):
            site_any.setdefault(j, (b, s))

    kwin = []   # per bin: (a, K) conv1 contraction window
    for bs in binsites:
        cont = set()
        for j in bs:
            cont.update(contributors(j))
        lo, hi = min(cont), max(cont)
        a = lo // 32
        K = (hi // 32 - a + 1) * 32
        assert K <= 128
        kwin.append((a, K))

    # extras for n=5 sites: 5th neighbor read from whatever bin holds it
    extra = {}
    for i in range(S):
        if len(nbrs[i]) == 5:
            k5, j5 = nbrs[i][4]
            extra[i] = (k5,) + site_any[j5]   # (k, bin, slot)

    # ---- 2x2 stride-2 cells ------------------------------------------------
    cellmap = {}
    for j in range(S):
        y, x = int(yy_s[j]), int(xx_s[j])
        cellmap.setdefault((y // 2, x // 2), []).append(((y % 2) * 2 + (x % 2), j))
    cells = sorted(cellmap)
    cellsites = [cellmap[c] for c in cells]
    C2 = len(cells)

    # ---- H2S packing: site pairs on partition halves -----------------------
    site_place = {}
    nb2 = 0
    for lst in cellsites:
        if len(lst) >= 2:
            site_place[lst[0][1]] = (nb2, 0)
            site_place[lst[1][1]] = (nb2, 1)
            nb2 += 1
    singles = []
    for lst in cellsites:
        if len(lst) == 1:
            singles.append(lst[0][1])
        elif len(lst) == 3:
            singles.append(lst[2][1])
    assert len(singles) % 2 == 0, "odd singles need a partial-block path"
    for p in range(0, len(singles), 2):
        site_place[singles[p]] = (nb2, 0)
        site_place[singles[p + 1]] = (nb2, 1)
        nb2 += 1

    # w3stack columns for multi-site cells
    w3cols = {}
    n = 0
    for c in range(C2):
        if len(cellsites[c]) >= 2:
            w3cols[c] = n
            n += 1

    return dict(order=order, nbrs=nbrs, cells=cells, cellsites=cellsites, C2=C2,
                binsites=binsites, binslot=binslot, assign=assign, kwin=kwin,
                extra=extra, nb1=nb1, site_place=site_place, nb2=nb2,
                w3cols=w3cols, nw3=max(1, n))


# ----------------------------------------------------------- device program --

def _legalize_single_wait(bir_bytes):
    """Split instructions with >1 sem-wait into EventSemaphore + instruction.

    The walrus build in this environment supports a single sync-wait slot per
    instruction; Tile emits fused multi-waits. Carry the extra waits on
    standalone EventSemaphore instructions on the same engine (same semantics:
    the engine blocks in order until each condition passes).
    """
    import json as _json
    bir = _json.loads(bir_bytes)
    ctr = 0
    for fn in bir.get("functions", []):
        for blk in fn.get("blocks", []):
            insts = blk.get("instructions")
            if not insts:
                continue
            out = []
            for inst in insts:
                si = inst.get("sync_info")
                waits = (si or {}).get("on_wait") or []
                if len(waits) > 1:
                    for wt in waits[:-1]:
                        ctr += 1
                        out.append({
                            "debug": inst.get("debug", 0),
                            "engine": inst["engine"],
                            "ins": [], "outs": [],
                            "name": f"xw{ctr}_{inst['name']}",
                            "opcode": "EventSemaphore",
                            "sync_info": {"on_update": [], "on_wait": [wt]},
                        })
                    si["on_wait"] = [waits[-1]]
                out.append(inst)
            blk["instructions"] = out
    return _json.dumps(bir).encode()


def _build_program(meta):
    import concourse.bass as bass
    import concourse.mybir as mybir
    import concourse.tile as tile

    class _Bass(bass.Bass):
        def to_json_bytes(self):
            return _legalize_single_wait(super().to_json_bytes())

    dt = mybir.dt
    f32, bf16 = dt.float32, dt.bfloat16
    Relu = mybir.ActivationFunctionType.Relu
    Exp = mybir.ActivationFunctionType.Exp
    Ln = mybir.ActivationFunctionType.Ln
    add_op = mybir.AluOpType.add
    max_op = mybir.AluOpType.max
    X_axis = mybir.AxisListType.X

    nbrs, cellsites, C2 = meta["nbrs"], meta["cellsites"], meta["C2"]
    binslot, assign, kwin = meta["binslot"], meta["assign"], meta["kwin"]
    extra, site_place = meta["extra"], meta["site_place"]
    w3cols = meta["w3cols"]
    NB1, NB2, NW3 = meta["nb1"], meta["nb2"], meta["nw3"]
    NB3 = (C2 + 1) // 2

    nc = _Bass()
    # xc + t1p consolidated into one wide-row param: DMA arbitration between
    # queues is packet-proportional, and packet size == contiguous row run,
    # so the conv1-critical data must present large packets to win bandwidth
    # against the bulk weight streams.
    CRIT_COLS = 5 * BC + NB1 * 128
    p_crit = nc.declare_dram_parameter("crit", [128, CRIT_COLS], bf16, isOutput=False)
    p_w2s = nc.declare_dram_parameter("w2stack", [128, S * 64], bf16, isOutput=False)
    p_w2x = nc.declare_dram_parameter("w2x", [128, max(1, len(extra)) * 64],
                                      bf16, isOutput=False)
    p_w3s = nc.declare_dram_parameter("w3stack", [128, NW3 * 64], bf16, isOutput=False)
    p_w3 = nc.declare_dram_parameter("w3sg", [128, 8 * 64], bf16, isOutput=False)
    p_f1 = nc.declare_dram_parameter("fc1g", [128, NB3 * 128], bf16, isOutput=False)
    p_sm = nc.declare_dram_parameter("smalls", [128, 24], f32, isOutput=False)
    p_out = nc.declare_dram_parameter("out", [BC, 10], f32, isOutput=True)

    with tile.TileContext(nc) as tc:
        with (
            tc.tile_pool(name="consts", bufs=1) as consts,
            tc.tile_pool(name="acts", bufs=1) as acts,
            tc.tile_pool(name="pp", bufs=3, space=bass.MemorySpace.PSUM) as pp,
            tc.tile_pool(name="pfc", bufs=1, space=bass.MemorySpace.PSUM) as pfc,
            tc.tile_pool(name="small", bufs=2) as small,
        ):
            # ---- PE warm-up + ACT table preload during the DMA window -----
            wsrc = consts.tile([128, 256], bf16)
            nc.vector.memset(wsrc, 0.001)
            wps = pp.tile([128, 1024], f32, tag="ps")
            for w in range(NWARM):
                nc.tensor.matmul(wps[:, (w % 4) * 256:(w % 4) * 256 + 256],
                                 wsrc[:, 0:128], wsrc, start=True, stop=True)
            wact = small.tile([128, 1], f32, tag="wact")
            nc.scalar.activation(out=wact, in_=wsrc[:, 0:1], func=Relu)
            nc.scalar.activation(out=wact, in_=wact, func=Exp)
            nc.scalar.activation(out=wact, in_=wact, func=Ln)

            crit = consts.tile([128, CRIT_COLS], bf16)
            xc = crit[:, 0:5 * BC]
            w2stack = consts.tile([128, S * 64], bf16)
            w2x = consts.tile([128, max(1, len(extra)) * 64], bf16)
            w3stack = consts.tile([128, NW3 * 64], bf16)
            w3sg = consts.tile([128, 8 * 64], bf16)
            smalls = consts.tile([128, 24], f32)
            fc1g = consts.tile([128, NB3 * 128], bf16)

            b1t = smalls[:, 0:1]
            b2t = smalls[:, 1:2]
            b3t = smalls[:, 2:3]
            fc1bt = smalls[:, 3:4]
            fc2bb = smalls[:, 4:14]
            fc2w = smalls[:, 14:24]

            h1s = acts.tile([128, NB1 * BC], bf16)
            h2s = acts.tile([128, NB2 * BC], bf16)
            h3s = acts.tile([128, NB3 * BC], bf16)
            zt = acts.tile([128, BC], f32)

            # input DMA: conv1-critical data exclusively on the sync queue
            # in priority order (queues share the DMA engines, but a queue
            # ramps up over ~2us, so the big weights on the scalar queue
            # leave the early window to xc/t1p).  gpsimd SWDGE is unused:
            # it is slow and adds drain cost at teardown.
            crit_split = 5 * BC + (NB1 // 2) * 128
            nc.sync.dma_start(out=smalls, in_=p_sm[:])
            nc.sync.dma_start(out=crit[:, 0:crit_split],
                              in_=p_crit[:, 0:crit_split])
            nc.sync.dma_start(out=crit[:, crit_split:CRIT_COLS],
                              in_=p_crit[:, crit_split:CRIT_COLS])
            nc.scalar.dma_start(out=w2stack, in_=p_w2s[:], max_dma_last_dim=1024)
            nc.scalar.dma_start(out=w2x, in_=p_w2x[:])
            nc.scalar.dma_start(out=w3stack, in_=p_w3s[:], max_dma_last_dim=1024)
            nc.scalar.dma_start(out=w3sg, in_=p_w3[:])
            nc.scalar.dma_start(out=fc1g, in_=p_f1[:], max_dma_last_dim=1024)

            evac_ctr = [0]

            def evac(dst, src, bias):
                # dst = relu(src + bias); alternate engines to split the load
                if evac_ctr[0] % 2 == 0:
                    nc.scalar.activation(out=dst, in_=src, func=Relu,
                                         bias=bias, scale=1.0)
                else:
                    nc.vector.tensor_scalar(out=dst, in0=src, scalar1=bias,
                                            scalar2=0.0, op0=add_op, op1=max_op)
                evac_ctr[0] += 1

            def t1ap(b):
                """lhsT AP for conv1 bin b inside the crit tile."""
                off = 5 * BC + b * 128
                return crit[:, off:off + 128]

            # ---- conv1: H1S = relu(T1S^T @ Xwin + b1) ---------------------
            # one matmul per bin; 4 bins per PSUM supertile; K padded to 128
            # (zero operator rows) for the fast-weight-load path
            for t in range((NB1 + 3) // 4):
                bs = [b for b in range(4 * t, min(4 * t + 4, NB1))]
                ps = pp.tile([128, 1024], f32, tag="ps")
                for g, b in enumerate(bs):
                    a, _K = kwin[b]
                    nc.tensor.matmul(ps[:, g * 256:g * 256 + 256],
                                     t1ap(b),
                                     xc[:, a * BC:(a + 1) * BC],
                                     start=True, stop=True,
                                     tile_position=(0, 0))
                evac(h1s[:, bs[0] * BC:(bs[0] + len(bs)) * BC],
                     ps[:, 0:256 * len(bs)], b1t)

            # ---- conv2: one K=128 matmul per site (+extra for n=5) --------
            blocksites = [[] for _ in range(NB2)]
            for j, (b2, hf) in site_place.items():
                blocksites[b2].append((hf, j))

            def conv2_block(ps, t, g):
                for hf, i in sorted(blocksites[t]):
                    pb = assign[i]
                    has_extra = i in extra
                    nc.tensor.matmul(
                        ps[64 * hf:64 * hf + 64, g * 256:g * 256 + 256],
                        w2stack[:, i * 64:(i + 1) * 64],
                        h1s[:, pb * BC:(pb + 1) * BC],
                        start=True, stop=not has_extra,
                        tile_position=(0, 64 * hf))
                    if has_extra:
                        e = list(extra).index(i)
                        eb = extra[i][1]
                        nc.tensor.matmul(
                            ps[64 * hf:64 * hf + 64, g * 256:g * 256 + 256],
                            w2x[:, e * 64:(e + 1) * 64],
                            h1s[:, eb * BC:(eb + 1) * BC],
                            start=False, stop=True,
                            tile_position=(0, 64 * hf))

            for t in range((NB2 + 3) // 4):
                bs = list(range(4 * t, min(4 * t + 4, NB2)))
                ps = pp.tile([128, 1024], f32, tag="ps")
                for g, b in enumerate(bs):
                    conv2_block(ps, b, g)
                evac(h2s[:, bs[0] * BC:(bs[0] + len(bs)) * BC],
                     ps[:, 0:256 * len(bs)], b2t)

            # ---- conv3: one stacked matmul per cell (+extra for m=3) ------
            def conv3_cell(ps, c, g):
                hc = c % 2
                lst = cellsites[c]
                m = len(lst)
                chunks = []
                if m >= 2:
                    b2 = site_place[lst[0][1]][0]
                    chunks.append((w3stack[:, w3cols[c] * 64:(w3cols[c] + 1) * 64], b2))
                    if m == 3:
                        b2e, hf2 = site_place[lst[2][1]]
                        k3 = lst[2][0]
                        chunks.append((w3sg[:, (hf2 * 4 + k3) * 64:
                                            (hf2 * 4 + k3 + 1) * 64], b2e))
                else:
                    b2e, hf2 = site_place[lst[0][1]]
                    k3 = lst[0][0]
                    chunks.append((w3sg[:, (hf2 * 4 + k3) * 64:
                                        (hf2 * 4 + k3 + 1) * 64], b2e))
                for idx, (wap, b2) in enumerate(chunks):
                    nc.tensor.matmul(
                        ps[64 * hc:64 * hc + 64, g * 256:g * 256 + 256],
                        wap,
                        h2s[:, b2 * BC:(b2 + 1) * BC],
                        start=(idx == 0), stop=(idx == len(chunks) - 1),
                        tile_position=(0, 64 * hc))

            for t in range((NB3 + 3) // 4):
                ts = list(range(4 * t, min(4 * t + 4, NB3)))
                ps = pp.tile([128, 1024], f32, tag="ps")
                ncells_last = 0
                for g, tt in enumerate(ts):
                    for c in (2 * tt, 2 * tt + 1):
                        if c < C2:
                            conv3_cell(ps, c, g)
                            if tt == ts[-1]:
                                ncells_last += 1
                if ncells_last == 2:
                    evac(h3s[:, ts[0] * BC:(ts[-1] + 1) * BC],
                         ps[:, 0:256 * len(ts)], b3t)
                else:
                    # final cell count is odd: avoid evacuating stale rows
                    if len(ts) > 1:
                        evac(h3s[:, ts[0] * BC:ts[-1] * BC],
                             ps[:, 0:256 * (len(ts) - 1)], b3t)
                    evac(h3s[:64, ts[-1] * BC:(ts[-1] + 1) * BC],
                         ps[:64, 256 * (len(ts) - 1):256 * len(ts)],
                         smalls[:64, 2:3])

            # ---- FC1: z = relu(fc1g^T @ h3s + fc1_b) ----------------------
            psz = pfc.tile([128, BC], f32, tag="psz")
            for t in range(NB3):
                kt = min(128, C2 * 64 - t * 128)
                nc.tensor.matmul(psz, fc1g[:kt, t * 128:(t + 1) * 128],
                                 h3s[:kt, t * BC:(t + 1) * BC],
                                 start=(t == 0), stop=(t == NB3 - 1))
            nc.vector.tensor_scalar(out=zt, in0=psz, scalar1=fc1bt,
                                    scalar2=0.0, op0=add_op, op1=max_op)

            # ---- FC2 + log_softmax (batch on partitions) ------------------
            # |logits| < 0.5 for this input distribution: skip max-subtract
            for hb in range(2):
                psl = pfc.tile([128, 10], f32, tag="psl")
                nc.tensor.matmul(psl, zt[:, hb * 128:(hb + 1) * 128], fc2w,
                                 start=True, stop=True)
                u = small.tile([128, 10], f32, tag="u")
                nc.vector.tensor_add(u, psl, fc2bb)
                e = small.tile([128, 10], f32, tag="e")
                nc.scalar.activation(out=e, in_=u, func=Exp)
                sm = small.tile([128, 1], f32, tag="sm")
                nc.vector.reduce_sum(out=sm, in_=e, axis=X_axis)
                ls = small.tile([128, 1], f32, tag="ls")
                nc.scalar.activation(out=ls, in_=sm, func=Ln)
                o = small.tile([128, 10], f32, tag="o")
                nc.vector.tensor_scalar(out=o, in0=u, scalar1=ls, scalar2=0.0,
                                        op0=mybir.AluOpType.subtract,
                                        op1=mybir.AluOpType.bypass)
                nc.sync.dma_start(out=p_out[hb * 128:(hb + 1) * 128, :], in_=o)

    return nc


# ------------------------------------------------------------------- kernel --

def _fold_bn(w, g, b, m, v):
    s = np.asarray(g, np.float64) / np.sqrt(np.asarray(v, np.float64) + EPS)
    return (np.asarray(w, np.float64) * s).astype(np.float32), \
        (np.asarray(b, np.float64) - np.asarray(m, np.float64) * s).astype(np.float32)


def _host_arrays(meta, w1, g1, b1, m1, v1, w2, g2, b2, m2, v2,
                 w3, g3, b3, m3, v3, fc1_w, fc1_b, fc2_w, fc2_b):
    nbrs, cellsites, C2 = meta["nbrs"], meta["cellsites"], meta["C2"]
    binsites, binslot, kwin = meta["binsites"], meta["binslot"], meta["kwin"]
    assign, extra = meta["assign"], meta["extra"]
    NB1, NW3 = meta["nb1"], meta["nw3"]
    NB3 = (C2 + 1) // 2
    M1 = NB1 * 128

    w1f, t1 = _fold_bn(w1, g1, b1, m1, v1)
    w2f, t2 = _fold_bn(w2, g2, b2, m2, v2)
    w3f, t3 = _fold_bn(w3, g3, b3, m3, v3)

    # base conv1 operator columns per site: Tcols[src j, site, ch]
    w1k = w1f.reshape(9, 32)
    Tcols = np.zeros((S, S, 32), np.float32)
    for i in range(S):
        for k, j in nbrs[i]:
            Tcols[j, i] += w1k[k]

    # windowed stacked conv1 operator: bin b slot s holds site binsites[b][s]
    T1P = np.zeros((128, M1), np.float32)
    for b in range(NB1):
        a, _K = kwin[b]
        for s, j in enumerate(binsites[b]):
            cols = slice(b * 128 + s * 32, b * 128 + (s + 1) * 32)
            src = Tcols[32 * a: min(S, 32 * a + 128), j, :]
            T1P[:src.shape[0], cols] = src

    # conv2 stacked weights: site i's column over its bin's slot layout
    w2k = w2f.reshape(9, 32, 64)
    w2stack = np.zeros((128, S * 64), np.float32)
    for i in range(S):
        b = assign[i]
        for k, j in nbrs[i][:min(len(nbrs[i]), 4)]:
            s = binslot[b][j]
            w2stack[32 * s: 32 * (s + 1), i * 64:(i + 1) * 64] = w2k[k]
    w2x = np.zeros((128, max(1, len(extra)) * 64), np.float32)
    for e, i in enumerate(extra):
        k5, _eb, s5 = extra[i]
        w2x[32 * s5: 32 * (s5 + 1), e * 64:(e + 1) * 64] = w2k[k5]

    # conv3 weights: single-site table, one variant per (half, offset)
    w3k = w3f.reshape(4, 64, 64)
    w3sg = np.zeros((128, 8 * 64), np.float32)
    for hf in range(2):
        for k in range(4):
            w3sg[64 * hf:64 * hf + 64,
                 (hf * 4 + k) * 64:(hf * 4 + k + 1) * 64] = w3k[k]
    # stacked weights for multi-site cells (site0 half 0, site1 half 1)
    w3stack = np.zeros((128, NW3 * 64), np.float32)
    n = 0
    for c in range(C2):
        lst = cellsites[c]
        if len(lst) >= 2:
            for hf, (k3, _j) in enumerate(lst[:2]):
                w3stack[64 * hf:64 * hf + 64, n * 64:(n + 1) * 64] = w3k[k3]
            n += 1

    # FC1 rows gathered at active cells, (cell, ch) order, K-chunked
    fc1_w = np.asarray(fc1_w, np.float32)
    cells = meta["cells"]
    rows = np.zeros((NB3 * 128, 128), np.float32)
    for nn_, (cy, cx) in enumerate(cells):
        rows[nn_ * 64:(nn_ + 1) * 64] = fc1_w[np.arange(64) * 196 + cy * 14 + cx]
    fc1g = np.ascontiguousarray(
        rows.reshape(NB3, 128, 128).transpose(1, 0, 2).reshape(128, NB3 * 128))

    smalls = np.zeros((128, 24), np.float32)
    smalls[:, 0] = np.tile(t1, 4)
    smalls[:, 1] = np.tile(t2, 2)
    smalls[:, 2] = np.tile(t3, 2)
    smalls[:, 3] = np.asarray(fc1_b, np.float32)
    smalls[:, 4:14] = np.tile(np.asarray(fc2_b, np.float32), (128, 1))
    smalls[:, 14:24] = np.asarray(fc2_w, np.float32)

    arrs = {
        "w2stack": w2stack.astype(BF),
        "w2x": w2x.astype(BF),
        "w3stack": w3stack.astype(BF),
        "w3sg": w3sg.astype(BF),
        "fc1g": fc1g.astype(BF),
        "smalls": smalls,
        "_t1p": T1P.astype(BF),   # folded into per-core "crit" by kernel()
    }
    return arrs


def kernel(features, indices, batch_size, w1, g1, b1, m1, v1,
           w2, g2, b2, m2, v2, w3, g3, b3, m3, v3,
           fc1_w, fc1_b, fc2_w, fc2_b, _trace=False):
    from concourse.bass_utils import run_bass_kernel_spmd

    features = np.asarray(features, np.float32)
    indices = np.asarray(indices, np.int32)
    assert int(batch_size) == B and features.shape[0] == B * S

    assert np.array_equal(indices[:, 0], np.repeat(np.arange(B, dtype=np.int32), S)), \
        "indices must be batch-major"
    assert np.array_equal(indices[:, 1:].reshape(B, S, 2),
                          np.broadcast_to(indices[:S, 1:], (B, S, 2))), \
        "active pattern must be identical across the batch"

    yy, xx = indices[:S, 1].copy(), indices[:S, 2].copy()
    key = (yy.tobytes(), xx.tobytes())
    if key not in _CACHE:
        meta = _build_meta(yy, xx)
        _CACHE[key] = (meta, _build_program(meta))
    meta, nc = _CACHE[key]

    common = _host_arrays(meta, w1, g1, b1, m1, v1, w2, g2, b2, m2, v2,
                          w3, g3, b3, m3, v3, fc1_w, fc1_b, fc2_w, fc2_b)

    # X replicated at five 32-site alignments: copy a = sites [32a, 32a+128)
    XT = features.reshape(B, S)[:, meta["order"]].T  # [S, B]
    Xpad = np.zeros((32 * 4 + 128, B), np.float32)
    Xpad[:S] = XT
    t1p = common.pop("_t1p")
    in_maps = []
    for c in range(NCORES):
        m = dict(common)
        critc = np.zeros((128, 5 * BC + t1p.shape[1]), BF)
        for a in range(5):
            critc[:, a * BC:(a + 1) * BC] = Xpad[32 * a:32 * a + 128,
                                                 c * BC:(c + 1) * BC].astype(BF)
        critc[:, 5 * BC:] = t1p
        m["crit"] = critc
        in_maps.append(m)

    res = run_bass_kernel_spmd(nc, in_maps, list(range(NCORES)), trace=_trace)
    global LAST_RESULT
    LAST_RESULT = res
    out = np.concatenate([res.results[c]["out"] for c in range(NCORES)], axis=0)
    return np.asarray(out, np.float32)


LAST_RESULT = None


# revision 16
# speedup vs baseline: 1.1235x; 1.1235x over previous
"""Trainium2 Bass kernel for nn_NetV2 sparse CNN (submanifold sparse conv net).

Network: scatter 150 active pixels/image to 28x28 grid -> SubMConv3x3(1->32)+BN+ReLU
-> SubMConv3x3(32->64)+BN+ReLU -> SparseConv2x2s2(64->64)+BN+ReLU -> flatten(NCHW)
-> FC(12544->128)+ReLU -> FC(128->10) -> log_softmax.

Design notes:
  * The active-pixel pattern is identical for every image, so each sparse conv
    layer is a fixed gather+matmul structure shared batch-wide.
  * Batch is data-parallel across 8 cores (256 images/core) and lives on the
    matmul free (N) axis; channels/sites live on partitions. BN folds into
    conv weights+bias.
  * H1S blocks hold conv1 outputs as 4 x 32-channel site slots per 128-row
    block, packed with SLOT SHARING: a greedy set-cover places sites so that
    every output site's <=4 active neighbors live in ONE shared block (46
    blocks / 181 slots vs 338 slots for naive per-consumer stacking).  Each
    block is produced by a single windowed conv1 matmul (operator columns
    emit the site's 3x3-neighborhood combination), so conv2 is one K=128
    matmul per site into its PSUM half; n=5 sites accumulate one extra
    matmul against whichever block holds the 5th neighbor.
  * conv3 cells read stacked site pairs from H2S blocks; single-site cells
    use per-(half,offset) weight variants. All matmuls use tile_position row
    base 0, so PSUM banks can be packed freely.
  * PSUM is organized as 3 double-bank supertiles [128,1024]: 4 column
    groups of matmuls per supertile, evacuated by ONE fused relu+bias op
    alternating between the scalar and vector engines (PSUM reads are the
    1 elem/cycle/lane bottleneck; bigger ops amortize ~200ns/op overhead).
  * Input DMA is spread across the sync/scalar/gpsimd descriptor queues so
    xc+t1p land ASAP; small parameters are batched into one transfer.
  * Dummy warm-up matmuls + ACT-table preloads run during the DMA window so
    the PE HAM clock ramps to 2.4 GHz while inputs land.
  * FC1 only needs rows of fc1_w at active output cells (117 of 196).
  * log_softmax skips the max-subtraction: |logits| < 0.5 by construction
    of the fixed input distribution.

All matmul operands are bf16 (fp32 PSUM accumulate); rel err ~2e-4 vs fp32 ref.
"""

import numpy as np
import ml_dtypes

B = 2048
S = 150          # active sites per image
H = W = 28
NCORES = 8
BC = B // NCORES  # batch per core = 256
EPS = 1e-5
BF = ml_dtypes.bfloat16
NWARM = 16       # PE warm-up matmuls


_CACHE = {}


# ---------------------------------------------------------------- metadata ---

def _build_meta(yy, xx):
    """Site graph + placement metadata from the shared active-pixel pattern."""
    order = np.argsort(yy.astype(np.int64) * W + xx)  # row-major spatial sort
    yy_s, xx_s = yy[order], xx[order]
    grid = -np.ones((H, W), np.int64)
    grid[yy_s, xx_s] = np.arange(S)

    # 3x3 pad-1 neighbor lists: per out site i, list of (k, j)
    nbrs = []
    for i in range(S):
        y, x = int(yy_s[i]), int(xx_s[i])
        lst = []
        for ky in range(3):
            for kx in range(3):
                iy, ix = y + ky - 1, x + kx - 1
                if 0 <= iy < H and 0 <= ix < W and grid[iy, ix] >= 0:
                    lst.append((ky * 3 + kx, int(grid[iy, ix])))
        nbrs.append(lst)

    def contributors(j):
        # conv1 column for site j sums over j's own 3x3 active neighbors
        return [j2 for _, j2 in nbrs[j]]

    def window_ok(sites):
        cont = set()
        for j in sites:
            cont.update(contributors(j))
        lo, hi = min(cont), max(cont)
        return (hi // 32 - lo // 32) * 32 + 32 <= 128

    # ---- H1S shared-slot packing -------------------------------------------
    # Bins of <=4 sites; every output site's prim4 neighbor set must fit in
    # one bin whose conv1 contributor-union spans <=128 sorted-site window.
    prim4 = [frozenset(j for _, j in nbrs[i][:min(len(nbrs[i]), 4)])
             for i in range(S)]
    import random as _random
    rnd = _random.Random(0)

    def pack(site_order):
        bins, assign = [], {}
        for i in site_order:
            sset = prim4[i]
            hit = False
            for b, bset in enumerate(bins):
                if sset <= bset:
                    assign[i] = b
                    hit = True
                    break
            if hit:
                continue
            best = None
            for b, bset in enumerate(bins):
                u = bset | sset
                if len(u) <= 4 and window_ok(u):
                    cost = len(u) - len(bset)
                    if best is None or cost < best[0]:
                        best = (cost, b, u)
            if best is not None:
                _, b, u = best
                bins[b] = u
                assign[i] = b
            else:
                bins.append(set(sset))
                assign[i] = len(bins) - 1
        return bins, assign

    best = None
    orders = [sorted(range(S), key=lambda i: (-len(prim4[i]), i))]
    for _ in range(30):
        orders.append(sorted(range(S), key=lambda i: (-len(prim4[i]),
                                                      rnd.random())))
    for o in orders:
        bins, assign = pack(o)
        if best is None or len(bins) < best[0]:
            best = (len(bins), bins, assign)
    nb1, bins, assign = best
    binsites = [sorted(bset) for bset in bins]      # slot s -> site
    binslot = [{j: s for s, j in enumerate(bs)} for bs in binsites]
    site_any = {}                                   # site -> (bin, slot)
    for b, bs in enumerate(binsites):
        for s, j in enumerate(bs):
            site_any.setdefault(j, (b, s))

    kwin = []   # per bin: (a, K) conv1 contraction window
    for bs in binsites:
        cont = set()
        for j in bs:
            cont.update(contributors(j))
        lo, hi = min(cont), max(cont)
        a = lo // 32
        K = (hi // 32 - a + 1) * 32
        assert K <= 128
        kwin.append((a, K))

    # extras for n=5 sites: 5th neighbor read from whatever bin holds it
    extra = {}
    for i in range(S):
        if len(nbrs[i]) == 5:
            k5, j5 = nbrs[i][4]
            extra[i] = (k5,) + site_any[j5]   # (k, bin, slot)

    # ---- 2x2 stride-2 cells ------------------------------------------------
    cellmap = {}
    for j in range(S):
        y, x = int(yy_s[j]), int(xx_s[j])
        cellmap.setdefault((y // 2, x // 2), []).append(((y % 2) * 2 + (x % 2), j))
    cells = sorted(cellmap)
    cellsites = [cellmap[c] for c in cells]
    C2 = len(cells)

    # ---- H2S packing: site pairs on partition halves -----------------------
    site_place = {}
    nb2 = 0
    for lst in cellsites:
        if len(lst) >= 2:
            site_place[lst[0][1]] = (nb2, 0)
            site_place[lst[1][1]] = (nb2, 1)
            nb2 += 1
    singles = []
    for lst in cellsites:
        if len(lst) == 1:
            singles.append(lst[0][1])
        elif len(lst) == 3:
            singles.append(lst[2][1])
    assert len(singles) % 2 == 0, "odd singles need a partial-block path"
    for p in range(0, len(singles), 2):
        site_place[singles[p]] = (nb2, 0)
        site_place[singles[p + 1]] = (nb2, 1)
        nb2 += 1

    # w3stack columns for multi-site cells
    w3cols = {}
    n = 0
    for c in range(C2):
        if len(cellsites[c]) >= 2:
            w3cols[c] = n
            n += 1

    return dict(order=order, nbrs=nbrs, cells=cells, cellsites=cellsites, C2=C2,
                binsites=binsites, binslot=binslot, assign=assign, kwin=kwin,
                extra=extra, nb1=nb1, site_place=site_place, nb2=nb2,
                w3cols=w3cols, nw3=max(1, n))


# ----------------------------------------------------------- device program --

def _legalize_single_wait(bir_bytes):
    """Split instructions with >1 sem-wait into EventSemaphore + instruction.

    The walrus build in this environment supports a single sync-wait slot per
    instruction; Tile emits fused multi-waits. Carry the extra waits on
    standalone EventSemaphore instructions on the same engine (same semantics:
    the engine blocks in order until each condition passes).
    """
    import json as _json
    bir = _json.loads(bir_bytes)
    ctr = 0
    for fn in bir.get("functions", []):
        for blk in fn.get("blocks", []):
            insts = blk.get("instructions")
            if not insts:
                continue
            out = []
            for inst in insts:
                si = inst.get("sync_info")
                waits = (si or {}).get("on_wait") or []
                if len(waits) > 1:
                    for wt in waits[:-1]:
                        ctr += 1
                        out.append({
                            "debug": inst.get("debug", 0),
                            "engine": inst["engine"],
                            "ins": [], "outs": [],
                            "name": f"xw{ctr}_{inst['name']}",
                            "opcode": "EventSemaphore",
                            "sync_info": {"on_update": [], "on_wait": [wt]},
                        })
                    si["on_wait"] = [waits[-1]]
                out.append(inst)
            blk["instructions"] = out
    return _json.dumps(bir).encode()


def _build_program(meta):
    import concourse.bass as bass
    import concourse.mybir as mybir
    import concourse.tile as tile

    class _Bass(bass.Bass):
        def to_json_bytes(self):
            return _legalize_single_wait(super().to_json_bytes())

    dt = mybir.dt
    f32, bf16 = dt.float32, dt.bfloat16
    Relu = mybir.ActivationFunctionType.Relu
    Exp = mybir.ActivationFunctionType.Exp
    Ln = mybir.ActivationFunctionType.Ln
    add_op = mybir.AluOpType.add
    max_op = mybir.AluOpType.max
    X_axis = mybir.AxisListType.X

    nbrs, cellsites, C2 = meta["nbrs"], meta["cellsites"], meta["C2"]
    binslot, assign, kwin = meta["binslot"], meta["assign"], meta["kwin"]
    extra, site_place = meta["extra"], meta["site_place"]
    w3cols = meta["w3cols"]
    NB1, NB2, NW3 = meta["nb1"], meta["nb2"], meta["nw3"]
    NB3 = (C2 + 1) // 2

    nc = _Bass()
    # xc + t1p consolidated into one wide-row param: DMA arbitration between
    # queues is packet-proportional, and packet size == contiguous row run,
    # so the conv1-critical data must present large packets to win bandwidth
    # against the bulk weight streams.
    CRIT_COLS = 5 * BC + NB1 * 128
    p_crit = nc.declare_dram_parameter("crit", [128, CRIT_COLS], bf16, isOutput=False)
    p_w2s = nc.declare_dram_parameter("w2stack", [128, S * 64], bf16, isOutput=False)
    p_w2x = nc.declare_dram_parameter("w2x", [128, max(1, len(extra)) * 64],
                                      bf16, isOutput=False)
    p_w3s = nc.declare_dram_parameter("w3stack", [128, NW3 * 64], bf16, isOutput=False)
    p_w3 = nc.declare_dram_parameter("w3sg", [128, 8 * 64], bf16, isOutput=False)
    p_f1 = nc.declare_dram_parameter("fc1g", [128, NB3 * 128], bf16, isOutput=False)
    p_sm = nc.declare_dram_parameter("smalls", [128, 24], f32, isOutput=False)
    p_out = nc.declare_dram_parameter("out", [BC, 10], f32, isOutput=True)

    with tile.TileContext(nc) as tc:
        with (
            tc.tile_pool(name="consts", bufs=1) as consts,
            tc.tile_pool(name="acts", bufs=1) as acts,
            tc.tile_pool(name="pp", bufs=3, space=bass.MemorySpace.PSUM) as pp,
            tc.tile_pool(name="pfc", bufs=1, space=bass.MemorySpace.PSUM) as pfc,
            tc.tile_pool(name="small", bufs=2) as small,
        ):
            # ---- PE warm-up + ACT table preload during the DMA window -----
            wsrc = consts.tile([128, 256], bf16)
            nc.vector.memset(wsrc, 0.001)
            wps = pp.tile([128, 1024], f32, tag="ps")
            for w in range(NWARM):
                nc.tensor.matmul(wps[:, (w % 4) * 256:(w % 4) * 256 + 256],
                                 wsrc[:, 0:128], wsrc, start=True, stop=True)
            wact = small.tile([128, 1], f32, tag="wact")
            nc.scalar.activation(out=wact, in_=wsrc[:, 0:1], func=Relu)
            nc.scalar.activation(out=wact, in_=wact, func=Exp)
            nc.scalar.activation(out=wact, in_=wact, func=Ln)

            crit = consts.tile([128, CRIT_COLS], bf16)
            xc = crit[:, 0:5 * BC]
            w2stack = consts.tile([128, S * 64], bf16)
            w2x = consts.tile([128, max(1, len(extra)) * 64], bf16)
            w3stack = consts.tile([128, NW3 * 64], bf16)
            w3sg = consts.tile([128, 8 * 64], bf16)
            smalls = consts.tile([128, 24], f32)
            fc1g = consts.tile([128, NB3 * 128], bf16)

            b1t = smalls[:, 0:1]
            b2t = smalls[:, 1:2]
            b3t = smalls[:, 2:3]
            fc1bt = smalls[:, 3:4]
            fc2bb = smalls[:, 4:14]
            fc2w = smalls[:, 14:24]

            h1s = acts.tile([128, NB1 * BC], bf16)
            h2s = acts.tile([128, NB2 * BC], bf16)
            h3s = acts.tile([128, NB3 * BC], bf16)
            zt = acts.tile([128, BC], f32)

            # input DMA: conv1-critical data exclusively on the sync queue
            # in priority order (queues share the DMA engines, but a queue
            # ramps up over ~2us, so the big weights on the scalar queue
            # leave the early window to xc/t1p).  gpsimd SWDGE is unused:
            # it is slow and adds drain cost at teardown.
            # One queue, strict priority order: DMA engines drain packets in
            # global issue order, so queue-level parallelism only lets the
            # bulk weights steal bandwidth from the conv1-critical data.
            crit_split = 5 * BC + (NB1 // 2) * 128
            nc.sync.dma_start(out=smalls, in_=p_sm[:])
            nc.sync.dma_start(out=crit[:, 0:crit_split],
                              in_=p_crit[:, 0:crit_split])
            nc.sync.dma_start(out=crit[:, crit_split:CRIT_COLS],
                              in_=p_crit[:, crit_split:CRIT_COLS])
            nc.sync.dma_start(out=w2stack, in_=p_w2s[:])
            nc.sync.dma_start(out=w2x, in_=p_w2x[:])
            nc.sync.dma_start(out=w3stack, in_=p_w3s[:])
            nc.sync.dma_start(out=w3sg, in_=p_w3[:])
            nc.sync.dma_start(out=fc1g, in_=p_f1[:])

            evac_ctr = [0]

            def evac(dst, src, bias):
                # dst = relu(src + bias); alternate engines to split the load
                if evac_ctr[0] % 2 == 0:
                    nc.scalar.activation(out=dst, in_=src, func=Relu,
                                         bias=bias, scale=1.0)
                else:
                    nc.vector.tensor_scalar(out=dst, in0=src, scalar1=bias,
                                            scalar2=0.0, op0=add_op, op1=max_op)
                evac_ctr[0] += 1

            def t1ap(b):
                """lhsT AP for conv1 bin b inside the crit tile."""
                off = 5 * BC + b * 128
                return crit[:, off:off + 128]

            # ---- conv1: H1S = relu(T1S^T @ Xwin + b1) ---------------------
            # one matmul per bin; 4 bins per PSUM supertile; K padded to 128
            # (zero operator rows) for the fast-weight-load path
            for t in range((NB1 + 3) // 4):
                bs = [b for b in range(4 * t, min(4 * t + 4, NB1))]
                ps = pp.tile([128, 1024], f32, tag="ps")
                for g, b in enumerate(bs):
                    a, _K = kwin[b]
                    nc.tensor.matmul(ps[:, g * 256:g * 256 + 256],
                                     t1ap(b),
                                     xc[:, a * BC:(a + 1) * BC],
                                     start=True, stop=True,
                                     tile_position=(0, 0))
                evac(h1s[:, bs[0] * BC:(bs[0] + len(bs)) * BC],
                     ps[:, 0:256 * len(bs)], b1t)

            # ---- conv2: one K=128 matmul per site (+extra for n=5) --------
            blocksites = [[] for _ in range(NB2)]
            for j, (b2, hf) in site_place.items():
                blocksites[b2].append((hf, j))

            def conv2_block(ps, t, g):
                for hf, i in sorted(blocksites[t]):
                    pb = assign[i]
                    has_extra = i in extra
                    nc.tensor.matmul(
                        ps[64 * hf:64 * hf + 64, g * 256:g * 256 + 256],
                        w2stack[:, i * 64:(i + 1) * 64],
                        h1s[:, pb * BC:(pb + 1) * BC],
                        start=True, stop=not has_extra,
                        tile_position=(0, 64 * hf))
                    if has_extra:
                        e = list(extra).index(i)
                        eb = extra[i][1]
                        nc.tensor.matmul(
                            ps[64 * hf:64 * hf + 64, g * 256:g * 256 + 256],
                            w2x[:, e * 64:(e + 1) * 64],
                            h1s[:, eb * BC:(eb + 1) * BC],
                            start=False, stop=True,
                            tile_position=(0, 64 * hf))

            for t in range((NB2 + 3) // 4):
                bs = list(range(4 * t, min(4 * t + 4, NB2)))
                ps = pp.tile([128, 1024], f32, tag="ps")
                for g, b in enumerate(bs):
                    conv2_block(ps, b, g)
                evac(h2s[:, bs[0] * BC:(bs[0] + len(bs)) * BC],
                     ps[:, 0:256 * len(bs)], b2t)

            # ---- conv3: one stacked matmul per cell (+extra for m=3) ------
            def conv3_cell(ps, c, g):
                hc = c % 2
                lst = cellsites[c]
                m = len(lst)
                chunks = []
                if m >= 2:
                    b2 = site_place[lst[0][1]][0]
                    chunks.append((w3stack[:, w3cols[c] * 64:(w3cols[c] + 1) * 64], b2))
                    if m == 3:
                        b2e, hf2 = site_place[lst[2][1]]
                        k3 = lst[2][0]
                        chunks.append((w3sg[:, (hf2 * 4 + k3) * 64:
                                            (hf2 * 4 + k3 + 1) * 64], b2e))
                else:
                    b2e, hf2 = site_place[lst[0][1]]
                    k3 = lst[0][0]
                    chunks.append((w3sg[:, (hf2 * 4 + k3) * 64:
                                        (hf2 * 4 + k3 + 1) * 64], b2e))
                for idx, (wap, b2) in enumerate(chunks):
                    nc.tensor.matmul(
                        ps[64 * hc:64 * hc + 64, g * 256:g * 256 + 256],
                        wap,
                        h2s[:, b2 * BC:(b2 + 1) * BC],
                        start=(idx == 0), stop=(idx == len(chunks) - 1),
                        tile_position=(0, 64 * hc))

            for t in range((NB3 + 3) // 4):
                ts = list(range(4 * t, min(4 * t + 4, NB3)))
                ps = pp.tile([128, 1024], f32, tag="ps")
                ncells_last = 0
                for g, tt in enumerate(ts):
                    for c in (2 * tt, 2 * tt + 1):
                        if c < C2:
                            conv3_cell(ps, c, g)
                            if tt == ts[-1]:
                                ncells_last += 1
                if ncells_last == 2:
                    evac(h3s[:, ts[0] * BC:(ts[-1] + 1) * BC],
                         ps[:, 0:256 * len(ts)], b3t)
                else:
                    # final cell count is odd: avoid evacuating stale rows
                    if len(ts) > 1:
                        evac(h3s[:, ts[0] * BC:ts[-1] * BC],
                             ps[:, 0:256 * (len(ts) - 1)], b3t)
                    evac(h3s[:64, ts[-1] * BC:(ts[-1] + 1) * BC],
                         ps[:64, 256 * (len(ts) - 1):256 * len(ts)],
                         smalls[:64, 2:3])

            # ---- FC1: z = relu(fc1g^T @ h3s + fc1_b) ----------------------
            psz = pfc.tile([128, BC], f32, tag="psz")
            for t in range(NB3):
                kt = min(128, C2 * 64 - t * 128)
                nc.tensor.matmul(psz, fc1g[:kt, t * 128:(t + 1) * 128],
                                 h3s[:kt, t * BC:(t + 1) * BC],
                                 start=(t == 0), stop=(t == NB3 - 1))
            nc.vector.tensor_scalar(out=zt, in0=psz, scalar1=fc1bt,
                                    scalar2=0.0, op0=add_op, op1=max_op)

            # ---- FC2 + log_softmax (batch on partitions) ------------------
            # |logits| < 0.5 for this input distribution: skip max-subtract
            for hb in range(2):
                psl = pfc.tile([128, 10], f32, tag="psl")
                nc.tensor.matmul(psl, zt[:, hb * 128:(hb + 1) * 128], fc2w,
                                 start=True, stop=True)
                u = small.tile([128, 10], f32, tag="u")
                nc.vector.tensor_add(u, psl, fc2bb)
                e = small.tile([128, 10], f32, tag="e")
                nc.scalar.activation(out=e, in_=u, func=Exp)
                sm = small.tile([128, 1], f32, tag="sm")
                nc.vector.reduce_sum(out=sm, in_=e, axis=X_axis)
                ls = small.tile([128, 1], f32, tag="ls")
                nc.scalar.activation(out=ls, in_=sm, func=Ln)
                o = small.tile([128, 10], f32, tag="o")
                nc.vector.tensor_scalar(out=o, in0=u, scalar1=ls, scalar2=0.0,
                                        op0=mybir.AluOpType.subtract,
                                        op1=mybir.AluOpType.bypass)
                nc.sync.dma_start(out=p_out[hb * 128:(hb + 1) * 128, :], in_=o)

    return nc


# ------------------------------------------------------------------- kernel --

def _fold_bn(w, g, b, m, v):
    s = np.asarray(g, np.float64) / np.sqrt(np.asarray(v, np.float64) + EPS)
    return (np.asarray(w, np.float64) * s).astype(np.float32), \
        (np.asarray(b, np.float64) - np.asarray(m, np.float64) * s).astype(np.float32)


def _host_arrays(meta, w1, g1, b1, m1, v1, w2, g2, b2, m2, v2,
                 w3, g3, b3, m3, v3, fc1_w, fc1_b, fc2_w, fc2_b):
    nbrs, cellsites, C2 = meta["nbrs"], meta["cellsites"], meta["C2"]
    binsites, binslot, kwin = meta["binsites"], meta["binslot"], meta["kwin"]
    assign, extra = meta["assign"], meta["extra"]
    NB1, NW3 = meta["nb1"], meta["nw3"]
    NB3 = (C2 + 1) // 2
    M1 = NB1 * 128

    w1f, t1 = _fold_bn(w1, g1, b1, m1, v1)
    w2f, t2 = _fold_bn(w2, g2, b2, m2, v2)
    w3f, t3 = _fold_bn(w3, g3, b3, m3, v3)

    # base conv1 operator columns per site: Tcols[src j, site, ch]
    w1k = w1f.reshape(9, 32)
    Tcols = np.zeros((S, S, 32), np.float32)
    for i in range(S):
        for k, j in nbrs[i]:
            Tcols[j, i] += w1k[k]

    # windowed stacked conv1 operator: bin b slot s holds site binsites[b][s]
    T1P = np.zeros((128, M1), np.float32)
    for b in range(NB1):
        a, _K = kwin[b]
        for s, j in enumerate(binsites[b]):
            cols = slice(b * 128 + s * 32, b * 128 + (s + 1) * 32)
            src = Tcols[32 * a: min(S, 32 * a + 128), j, :]
            T1P[:src.shape[0], cols] = src

    # conv2 stacked weights: site i's column over its bin's slot layout
    w2k = w2f.reshape(9, 32, 64)
    w2stack = np.zeros((128, S * 64), np.float32)
    for i in range(S):
        b = assign[i]
        for k, j in nbrs[i][:min(len(nbrs[i]), 4)]:
            s = binslot[b][j]
            w2stack[32 * s: 32 * (s + 1), i * 64:(i + 1) * 64] = w2k[k]
    w2x = np.zeros((128, max(1, len(extra)) * 64), np.float32)
    for e, i in enumerate(extra):
        k5, _eb, s5 = extra[i]
        w2x[32 * s5: 32 * (s5 + 1), e * 64:(e + 1) * 64] = w2k[k5]

    # conv3 weights: single-site table, one variant per (half, offset)
    w3k = w3f.reshape(4, 64, 64)
    w3sg = np.zeros((128, 8 * 64), np.float32)
    for hf in range(2):
        for k in range(4):
            w3sg[64 * hf:64 * hf + 64,
                 (hf * 4 + k) * 64:(hf * 4 + k + 1) * 64] = w3k[k]
    # stacked weights for multi-site cells (site0 half 0, site1 half 1)
    w3stack = np.zeros((128, NW3 * 64), np.float32)
    n = 0
    for c in range(C2):
        lst = cellsites[c]
        if len(lst) >= 2:
            for hf, (k3, _j) in enumerate(lst[:2]):
                w3stack[64 * hf:64 * hf + 64, n * 64:(n + 1) * 64] = w3k[k3]
            n += 1

    # FC1 rows gathered at active cells, (cell, ch) order, K-chunked
    fc1_w = np.asarray(fc1_w, np.float32)
    cells = meta["cells"]
    rows = np.zeros((NB3 * 128, 128), np.float32)
    for nn_, (cy, cx) in enumerate(cells):
        rows[nn_ * 64:(nn_ + 1) * 64] = fc1_w[np.arange(64) * 196 + cy * 14 + cx]
    fc1g = np.ascontiguousarray(
        rows.reshape(NB3, 128, 128).transpose(1, 0, 2).reshape(128, NB3 * 128))

    smalls = np.zeros((128, 24), np.float32)
    smalls[:, 0] = np.tile(t1, 4)
    smalls[:, 1] = np.tile(t2, 2)
    smalls[:, 2] = np.tile(t3, 2)
    smalls[:, 3] = np.asarray(fc1_b, np.float32)
    smalls[:, 4:14] = np.tile(np.asarray(fc2_b, np.float32), (128, 1))
    smalls[:, 14:24] = np.asarray(fc2_w, np.float32)

    arrs = {
        "w2stack": w2stack.astype(BF),
        "w2x": w2x.astype(BF),
        "w3stack": w3stack.astype(BF),
        "w3sg": w3sg.astype(BF),
        "fc1g": fc1g.astype(BF),
        "smalls": smalls,
        "_t1p": T1P.astype(BF),   # folded into per-core "crit" by kernel()
    }
    return arrs


def kernel(features, indices, batch_size, w1, g1, b1, m1, v1,
           w2, g2, b2, m2, v2, w3, g3, b3, m3, v3,
           fc1_w, fc1_b, fc2_w, fc2_b, _trace=False):
    from concourse.bass_utils import run_bass_kernel_spmd

    features = np.asarray(features, np.float32)
    indices = np.asarray(indices, np.int32)
    assert int(batch_size) == B and features.shape[0] == B * S

    assert np.array_equal(indices[:, 0], np.repeat(np.arange(B, dtype=np.int32), S)), \
        "indices must be batch-major"
    assert np.array_equal(indices[:, 1:].reshape(B, S, 2),
                          np.broadcast_to(indices[:S, 1:], (B, S, 2))), \
        "active pattern must be identical across the batch"

    yy, xx = indices[:S, 1].copy(), indices[:S, 2].copy()
    key = (yy.tobytes(), xx.tobytes())
    if key not in _CACHE:
        meta = _build_meta(yy, xx)
        _CACHE[key] = (meta, _build_program(meta))
    meta, nc = _CACHE[key]

    common = _host_arrays(meta, w1, g1, b1, m1, v1, w2, g2, b2, m2, v2,
                          w3, g3, b3, m3, v3, fc1_w, fc1_b, fc2_w, fc2_b)

    # X replicated at five 32-site alignments: copy a = sites [32a, 32a+128)
    XT = features.reshape(B, S)[:, meta["order"]].T  # [S, B]
    Xpad = np.zeros((32 * 4 + 128, B), np.float32)
    Xpad[:S] = XT
    t1p = common.pop("_t1p")
    in_maps = []
    for c in range(NCORES):
        m = dict(common)
        critc = np.zeros((128, 5 * BC + t1p.shape[1]), BF)
        for a in range(5):
            critc[:, a * BC:(a + 1) * BC] = Xpad[32 * a:32 * a + 128,
                                                 c * BC:(c + 1) * BC].astype(BF)
        critc[:, 5 * BC:] = t1p
        m["crit"] = critc
        in_maps.append(m)

    res = run_bass_kernel_spmd(nc, in_maps, list(range(NCORES)), trace=_trace)
    global LAST_RESULT
    LAST_RESULT = res
    out = np.concatenate([res.results[c]["out"] for c in range(NCORES)], axis=0)
    return np.asarray(out, np.float32)


LAST_RESULT = None
